# revision 42
# baseline (speedup 1.0000x reference)
"""GCN layer (gnn_message_passing) on 8 Trainium2 NeuronCores.

Reference computation:
    deg = segment_sum(ones, hs)              # in-degree of each node (rows hs)
    s   = deg ** -0.5
    agg[h] = sum over edges (h, t) of s[t] * feats[t]
    out = relu((s[:, None] * agg) @ W.T)

Distribution strategy (per the sharding hint):
  * Nodes are sharded across the 8 cores; edges are partitioned by
    destination (hs) so the segment-sum is core-local.  Nodes are dealt to
    (core, 128-node group) slots by descending degree, greedily balancing
    per-source-class degree sums across cores within each 1024-rank window,
    so per-(pair-group, class) edge counts are nearly identical across the
    8 cores and the shared SPMD gather sizes (max over cores, padded to
    whole 128-slot columns) carry ~2% padding.
  * feats is replicated to every core's HBM as bf16 with the *source*
    normalization deg_t^-1/2 folded in on the host, split into 4 tables of
    25000 rows so row ids fit dma_gather's int16 index format.  Each core
    batch-gathers the source rows it needs (thousands of rows per call to
    amortize the ~1us SWDGE fixed cost).  Gather calls use the TRUE edge
    count padded to whole 128-slot columns (so every readable G slot holds
    finite data - NaN garbage times a zero mask is still NaN), eliminating
    the per-block alignment padding of a quartile-bucketed layout.
  * Output groups are processed in PAIRS (256 destinations) to halve the
    per-window scheduling cost.  The SpMM runs per pair as
        agg_T[f, s] = sum_w  G_w[e, f]^T @ S_w[e, s]        (s in 0..255)
    over the 128-row windows w of the gathered stream containing the pair's
    edges.  S_w is built by one DVE tensor_scalar op:
        S_w[e, s] = (iota[s] == off[e]) * w[e]
    where off is the destination position within the pair (pad slots carry
    300, producing zero columns) and w = deg_h^-1/2 folds the *destination*
    normalization.  A window shared by two adjacent pairs is processed once
    per pair with different off columns, so no gather alignment is needed.
  * S tiles live in slab tiles (SCH windows per slab) rather than a
    rotating per-window pool, which lets the tile framework elide the
    per-build semaphore-wait instructions that would otherwise pace the DVE
    sequencer.  Per group, the 128x128 linear runs transposed
    (out2 = W^T_tile x msg), so outputs are stored transposed and batched
    (KST groups per DMA) with >=512B descriptors.  The linear+relu+store
    for pair p-1 is emitted after pair p's accumulation matmuls so the PE
    never head-of-line blocks on the PSUM->SBUF copy (which runs on the
    Activation engine).  Gather batches have ~uniform row counts with a
    tapered tail so the final compute drain is short.

Every core runs the identical program (SPMD); only the per-core index/meta
data differs.
"""

import numpy as np
import ml_dtypes

import concourse.bacc as bacc
import concourse.bass as bass
import concourse.mybir as mybir
import concourse.tile as tile
from concourse import bass_utils

N_N = 100000
N_E = 1600000
D = 128
N_CORES = 8
P = 128
GPC = -(-N_N // (N_CORES * P))  # 98 groups of 128 node slots per core
NPC = GPC * P  # 12544 node positions per core (12500 real)
NPAIR = GPC // 2  # 49 pair-groups of 256 destinations
NQ = 4  # source tables (dma_gather indices are int16)
QS = 25000  # rows per source table
TARGET_BATCHES = 16  # gather batches of ~uniform row count
SCH = 24  # S-slab chunk: windows per slab tile
KST = 14  # output groups per batched store

F32 = mybir.dt.float32
BF16 = mybir.dt.bfloat16
I16 = mybir.dt.int16
BFNP = ml_dtypes.bfloat16


class Plan:
    """Static (per-dataset) schedule shared by all cores."""

    def __init__(self, batches, num_idx, w0, w1):
        self.batches = batches  # (jp0, jp1) pair-group ranges
        self.num_idx = num_idx  # [NB, NQ]
        nb = len(batches)
        self.cols = -(-num_idx // P)
        self.icols = -(-num_idx // 16)
        self.colbase = np.zeros((nb, NQ), np.int64)
        self.icolbase = np.zeros((nb, NQ), np.int64)
        self.bcols = np.zeros(nb, np.int64)
        ic = 0
        for b in range(nb):
            cb = 0
            for q in range(NQ):
                self.colbase[b, q] = cb
                self.icolbase[b, q] = ic
                cb += int(self.cols[b, q])
                ic += int(self.icols[b, q])
            self.bcols[b] = cb
        self.icols_total = ic

        # pair columns: (jp, q, w) -> pc, and per-pair matmul list
        self.pairs = [[] for _ in range(NPAIR)]
        self.pairbase = np.zeros((NPAIR, NQ), np.int64)
        b_of = np.zeros(NPAIR, np.int64)
        for bi, (g0, g1) in enumerate(batches):
            b_of[g0:g1] = bi
        pc = 0
        for jp in range(NPAIR):
            b = int(b_of[jp])
            for q in range(NQ):
                self.pairbase[jp, q] = pc
                for w in range(int(w0[jp, q]), int(w1[jp, q])):
                    self.pairs[jp].append((int(self.colbase[b, q]) + w, pc))
                    pc += 1
        self.npairs = pc
        self.w0 = w0

    def key(self):
        return (
            tuple(g for b in self.batches for g in b),
            tuple(self.num_idx.reshape(-1).tolist()),
            tuple(self.w0.reshape(-1).tolist()),
            tuple(len(p) for p in self.pairs),
        )


def prep(edges):
    """Host bookkeeping; see module docstring.

    Returns (plan, idx16, off, wsc, sh_unused, core_of, pos_of).
    """
    hs = np.asarray(edges[0], dtype=np.int64)
    ts = np.asarray(edges[1], dtype=np.int64)
    deg = np.bincount(hs, minlength=N_N)
    sdi = (deg.astype(np.float64) ** -0.5).astype(np.float32)

    # Deal nodes by descending degree into 8 cores x GPC slots, greedily
    # balancing the per-source-class degree sums across cores within each
    # 1024-rank window (so gather sizes and window spans match across the
    # SPMD cores).
    order = np.argsort(-deg, kind="stable")
    eq_n = ts // QS
    dq = np.zeros((N_N, NQ), np.int64)
    np.add.at(dq, (hs, eq_n), 1)
    core_of = np.empty(N_N, np.int64)
    j_of = np.empty(N_N, np.int64)
    p_of = np.empty(N_N, np.int64)
    for w in range(GPC):
        nodes = order[w * 8 * P : (w + 1) * 8 * P]
        sums = np.zeros((N_CORES, NQ), np.int64)
        counts = np.zeros(N_CORES, np.int64)
        for r0 in range(0, len(nodes), 8):
            octet = sorted(
                nodes[r0 : r0 + 8].tolist(), key=lambda n: -deg[n]
            )
            for n in octet:
                cand = np.nonzero(counts < P)[0]
                score = (sums[cand] + dq[n]).max(axis=1) + 1e-3 * sums[
                    cand
                ].sum(axis=1)
                c = int(cand[int(np.argmin(score))])
                core_of[n] = c
                p_of[n] = counts[c]
                j_of[n] = w
                sums[c] += dq[n]
                counts[c] += 1

    ec = core_of[hs]
    ej = j_of[hs]
    ejp = ej // 2  # pair-group
    # destination position within the pair (0..255)
    epp = (ej - ejp * 2) * P + p_of[hs]
    eq = ts // QS
    etl = ts % QS

    # per-(c,jp,q) counts
    cnt = np.bincount(
        (ec * NPAIR + ejp) * NQ + eq, minlength=N_CORES * NPAIR * NQ
    ).reshape(N_CORES, NPAIR, NQ)

    # Edge-budget batches over pair-groups (~uniform max-core rows).
    pair_rows = cnt.max(axis=0).sum(axis=1)
    target = int(pair_rows.sum() / TARGET_BATCHES) + 1
    batches = []
    g0 = 0
    acc = 0
    for jp in range(NPAIR):
        acc += int(pair_rows[jp])
        # taper the final batches so the compute tail drains quickly
        t = target
        if jp >= NPAIR - 2:
            t = target // 6
        elif jp >= NPAIR - 5:
            t = target // 3
        elif jp >= NPAIR - 9:
            t = target // 2
        if acc >= t or jp == NPAIR - 1:
            batches.append((g0, jp + 1))
            g0 = jp + 1
            acc = 0
    NB = len(batches)
    b_of = np.zeros(NPAIR, np.int64)
    for bi, (bg0, bg1) in enumerate(batches):
        b_of[bg0:bg1] = bi
    eb = b_of[ejp]

    # prefix within batch (pair-major inside call)
    s0 = np.zeros_like(cnt)
    cnt_cbq = np.zeros((N_CORES, NB, NQ), np.int64)
    for b, (j0, j1) in enumerate(batches):
        c = np.cumsum(cnt[:, j0:j1, :], axis=1)
        s0[:, j0 + 1 : j1, :] = c[:, :-1, :]
        cnt_cbq[:, b, :] = cnt[:, j0:j1, :].sum(axis=1)
    # Pad gather sizes to whole 128-slot columns so every G slot the
    # matmuls can read is written (pad indices fetch table row 0 and are
    # masked by off=300); unwritten SBUF could hold NaNs, and NaN*0 = NaN.
    num_idx = -(-cnt_cbq.max(axis=0) // P) * P
    s1 = s0 + cnt

    w0 = s0.min(axis=0) // P  # [NPAIR, NQ]
    w1 = -(-(s1.max(axis=0)) // P)
    w1 = np.maximum(w1, w0)
    plan = Plan(batches, num_idx, w0, w1)

    # Edge stream order: (core, batch, class, pair, source)
    skey = ((ec * NB + eb) * NQ + eq) * NPAIR + ejp
    order_e = np.lexsort((ts, skey))
    skey_s = skey[order_e]
    starts = np.zeros(N_CORES * NB * NQ * NPAIR + 1, np.int64)
    np.cumsum(
        np.bincount(skey_s, minlength=N_CORES * NB * NQ * NPAIR),
        out=starts[1:],
    )
    rank = np.arange(N_E, dtype=np.int64) - starts[skey_s]
    ec_s = ec[order_e]
    ejp_s = ejp[order_e]
    eq_s = eq[order_e]
    i_s = s0[ec_s, ejp_s, eq_s] + rank
    w_s = i_s // P
    p_s = i_s % P

    idx16 = np.zeros((N_CORES, 16, plan.icols_total), np.int16)
    eb_s = b_of[ejp_s]
    icol = plan.icolbase[eb_s, eq_s] + i_s // 16
    idx16[ec_s, i_s % 16, icol] = etl[order_e].astype(np.int16)
    idx16 = np.tile(idx16, (1, 8, 1))

    # off/w: [cores, P, npairs] f32
    offa = np.full((N_CORES, P, plan.npairs), 300.0, np.float32)
    wa = np.zeros((N_CORES, P, plan.npairs), np.float32)
    pcol = plan.pairbase[ejp_s, eq_s] + (w_s - w0[ejp_s, eq_s])
    offa[ec_s, p_s, pcol] = epp[order_e]
    wa[ec_s, p_s, pcol] = sdi[hs[order_e]]

    pos_of = j_of * P + p_of
    return plan, idx16, offa, wa, core_of, pos_of


def build_gcn(plan, g_bufs=3, s_bufs=3, ps_a=4, ps_b=3):
    """Build the SPMD Bass program for one core (all cores identical)."""
    nc = bacc.Bacc(
        "TRN2",
        target_bir_lowering=False,
        debug=False,
        enable_asserts=False,
        num_devices=N_CORES,
    )
    fq_d = [
        nc.dram_tensor(f"f16q{q}", [QS, D], BF16, kind="ExternalInput")
        for q in range(NQ)
    ]
    idx_d = nc.dram_tensor(
        "idx16", [P, plan.icols_total], I16, kind="ExternalInput"
    )
    off_d = nc.dram_tensor("off", [P, plan.npairs], F32, kind="ExternalInput")
    w_d = nc.dram_tensor("w", [P, plan.npairs], F32, kind="ExternalInput")
    wt_d = nc.dram_tensor("wt", [P, P], BF16, kind="ExternalInput")
    iota_d = nc.dram_tensor("iota", [P, 2 * P], BF16, kind="ExternalInput")
    out_d = nc.dram_tensor("outT", [P, NPC], BF16, kind="ExternalOutput")

    with tile.TileContext(nc) as tc:
        with (
            tc.tile_pool(name="const", bufs=1) as cpool,
            tc.tile_pool(name="gpool", bufs=g_bufs) as gpool,
            tc.tile_pool(name="spool", bufs=s_bufs) as spool,
            tc.tile_pool(name="mpool", bufs=3) as mpool,
            tc.tile_pool(name="opool", bufs=2) as opool,
            tc.tile_pool(name="psA", bufs=ps_a, space="PSUM") as psA,
            tc.tile_pool(name="psB", bufs=ps_b, space="PSUM") as psB,
        ):
            wt_sb = cpool.tile([P, P], BF16)
            nc.sync.dma_start(wt_sb[:], wt_d[:])
            iota_sb = cpool.tile([P, 2 * P], BF16)
            nc.sync.dma_start(iota_sb[:], iota_d[:])
            off_sb = cpool.tile([P, plan.npairs], F32)
            nc.sync.dma_start(off_sb[:], off_d[:])
            w_sb = cpool.tile([P, plan.npairs], F32)
            nc.sync.dma_start(w_sb[:], w_d[:])
            idx_sb = cpool.tile([P, plan.icols_total], I16)
            nb = len(plan.batches)
            for b in range(nb):
                b0 = int(plan.icolbase[b, 0])
                b1 = (
                    int(plan.icolbase[b + 1, 0])
                    if b + 1 < nb
                    else plan.icols_total
                )
                if b1 > b0:
                    nc.sync.dma_start(idx_sb[:, b0:b1], idx_d[:, b0:b1])

            slab_starts = list(range(0, GPC - 14, KST)) + [GPC - 14, GPC - 7]
            slab_of = {}
            for si, s0_ in enumerate(slab_starts):
                s1_ = slab_starts[si + 1] if si + 1 < len(slab_starts) else GPC
                for j_ in range(s0_, s1_):
                    slab_of[j_] = (s0_, s1_ - s0_)
            state = {"slab": None, "pending": None, "wi": 0, "sslab": None}

            def tail2(jp, msgt):
                # linear + relu + (batched, transposed) store for pair jp
                for half in range(2):
                    j = jp * 2 + half
                    out2 = psB.tile([P, P], F32, tag="out2")
                    nc.tensor.matmul(
                        out2[:],
                        lhsT=wt_sb[:],
                        rhs=msgt[:, half * P : (half + 1) * P],
                        start=True,
                        stop=True,
                    )
                    s0_, slen = slab_of[j]
                    if j == s0_:
                        state["slab"] = opool.tile(
                            [P, slen * P], BF16, tag="oslab", name="oslab"
                        )
                    sl = j - s0_
                    nc.scalar.activation(
                        state["slab"][:, sl * P : (sl + 1) * P],
                        out2[:],
                        mybir.ActivationFunctionType.Relu,
                    )
                    if sl == slen - 1:
                        nc.sync.dma_start(
                            out_d[:, s0_ * P : (s0_ + slen) * P],
                            state["slab"][:],
                        )

            for b, (g0, g1) in enumerate(plan.batches):
                bcols = int(plan.bcols[b])
                Gt = gpool.tile([P, bcols, P], BF16, tag="G")
                for q in range(NQ):
                    num = int(plan.num_idx[b, q])
                    if num == 0:
                        continue
                    cb = int(plan.colbase[b, q])
                    ncol = int(plan.cols[b, q])
                    ic0 = int(plan.icolbase[b, q])
                    nic = int(plan.icols[b, q])
                    nc.gpsimd.dma_gather(
                        Gt[:, cb : cb + ncol, :],
                        fq_d[q][:],
                        idx_sb[:, ic0 : ic0 + nic],
                        num,
                        num,
                        D,
                        single_packet=False,
                    )

                for jp in range(g0, g1):
                    mms = plan.pairs[jp]
                    agg = psA.tile([P, 2 * P], F32, tag="agg")
                    for ki, (gcol, pc) in enumerate(mms):
                        wi = state["wi"] % SCH
                        if wi == 0:
                            state["sslab"] = spool.tile(
                                [P, SCH * 2 * P], BF16, tag="S", name="Sslab"
                            )
                        state["wi"] += 1
                        St = state["sslab"][:, wi * 2 * P : (wi + 1) * 2 * P]
                        nc.vector.tensor_scalar(
                            out=St,
                            in0=iota_sb[:],
                            scalar1=off_sb[:, pc : pc + 1],
                            scalar2=w_sb[:, pc : pc + 1],
                            op0=mybir.AluOpType.is_equal,
                            op1=mybir.AluOpType.mult,
                        )
                        nc.tensor.matmul(
                            agg[:],
                            lhsT=Gt[:, gcol, :],
                            rhs=St,
                            start=(ki == 0),
                            stop=(ki == len(mms) - 1),
                        )
                    # agg is [feat, 256 dests]; copy on Activation engine.
                    msgt = mpool.tile([P, 2 * P], BF16, tag="msgt")
                    nc.scalar.activation(
                        msgt[:], agg[:], mybir.ActivationFunctionType.Copy
                    )
                    if state["pending"] is not None:
                        tail2(*state["pending"])
                    state["pending"] = (jp, msgt)
            if state["pending"] is not None:
                tail2(*state["pending"])

    nc.compile()
    return nc


_CACHE = {}


def _run(feats_n, edges, weight, trace=False):
    feats = np.asarray(feats_n, dtype=np.float32)
    weight = np.asarray(weight, dtype=np.float32)
    plan, idx16, off, wa, core_of, pos_of = prep(edges)

    key = plan.key()
    if key not in _CACHE:
        _CACHE[key] = build_gcn(plan)
    nc = _CACHE[key]

    deg = np.bincount(np.asarray(edges[0], dtype=np.int64), minlength=N_N)
    sdi = (deg.astype(np.float64) ** -0.5).astype(np.float32)
    table = (sdi[:, None] * feats).astype(BFNP)
    fq = [
        np.ascontiguousarray(table[q * QS : (q + 1) * QS]) for q in range(NQ)
    ]
    wt = np.ascontiguousarray(weight.T).astype(BFNP)
    iota = np.ascontiguousarray(
        np.broadcast_to(np.arange(2 * P, dtype=BFNP), (P, 2 * P))
    )
    in_maps = [
        {
            **{f"f16q{q}": fq[q] for q in range(NQ)},
            "idx16": idx16[c],
            "off": off[c],
            "w": wa[c],
            "wt": wt,
            "iota": iota,
        }
        for c in range(N_CORES)
    ]
    res = bass_utils.run_bass_kernel_spmd(
        nc, in_maps, core_ids=list(range(N_CORES)), trace=trace
    )
    outs = [
        np.asarray(res.results[c]["outT"], dtype=np.float32)
        for c in range(N_CORES)
    ]
    out = np.empty((N_N, D), np.float32)
    for c in range(N_CORES):
        m = core_of == c
        out[m] = outs[c][:, pos_of[m]].T
    return out, res


def kernel(feats_n, edges, weight):
    out, _ = _run(feats_n, edges, weight)
    return out
